# revision 21
# baseline (speedup 1.0000x reference)
"""Biaffine edge attention on 8 Trainium2 NeuronCores (fp16, PE-roofline schedule).

Math (per batch b):
    out[i,o] = head[i,:] @ U @ dep[o,:] + head[i,:]@wh + dep[o,:]@wd + b
with head/dep [S=2048, D=256], U [D,D], edge_W = [wh | wd] (each [D]).

Sharding: pure data-parallel over batch B=8 -> one batch per core,
U / edge_W / edge_b replicated. No collectives.

Host prep (layout only + the tiny rank-1 bias):
    headT/depT: inputs pre-transposed to [D, S] fp16 and packed
        [dc0 | dc1] side-by-side as [128, 2S] -- the PE needs the
        contraction dim on partitions for both operands, host-side
        layout beats 64 PE transposes, and the single-DMA packing gives
        8KB DRAM elements (~340 GB/s vs ~220 at 2KB).
    hs2[p, j] = head[j*128+p, :] @ wh + b   (per-row bias, [128, 16] f32)
    u2 = [U[:128, :] | U[128:, :]]          ([128, 512] fp16)
    wd2[p, eb] = wd[eb*128+p]               ([128, 2] f32)

Per-core kernel (fp16 matmuls, f32 PSUM, fp16 stores upcast on host):
    ATf[e,i] = sum_d U[d,e] headT[d,i] + wd[e]    (ds[o] rides the
               e-contraction of the out matmul for free)
    out[i,o] = sum_e ATf[e,i] depT[e,o] + hs2[i]  (bias fused in the
               PSUM->SBUF eviction on ACT/DVE)

Schedule notes:
  - all loads FIFO-serialized on the ACT HWDGE ring in priority order
    (u2, head, dep) -- parallel rings would packet-interleave and delay
    the head bytes the ATf phase waits on; SP ring kept for stores.
  - ~7 junk matmuls on a memset tile warm the PE HAM clock gate
    (1.2 -> 2.4 GHz needs ~3.4us of sustained busy) during the loads.
  - fp16 moving operands stream N=1024 per matmul: out row-blocks are
    2 matmuls x 2 eb into two 2-bank PSUM tiles; halves the PE
    instruction count and the eviction count vs 512-chunks.
  - epilogue: one [128,1024] eviction on DVE + one on ACT per row;
    row bias (hs2) and ATf bias (wd2) ride the eviction for free.
  - rows 0-2 split in halves: their first halves only need the first
    half of dep, filling the PE window while the dep tail is still on
    the DMA ring; second halves run mid-stream.
  - stores: one [128,2048] fp16 DMA per row-block on the SP ring.
"""

import numpy as np

import concourse.bass as bass
import concourse.tile as tile
from concourse import bacc, mybir
from concourse.bass_utils import run_bass_kernel_spmd

B, S, D = 8, 2048, 256
P = 128          # partitions
NB = 1024        # matmul moving free-dim block (two PSUM banks of fp32)
NI = S // P      # 16 output row blocks
ND = D // P      # 2 contraction chunks
NWARM = 7        # PE warm-up filler matmuls (cover load latency, warm HAM)
F32 = mybir.dt.float32
F16 = mybir.dt.float16

Ident = mybir.ActivationFunctionType.Identity


def build_nc(reps=1):
    """reps>1 wraps the body in a HW For_i loop -- used only for timing."""
    nc = bacc.Bacc("TRN2", target_bir_lowering=False, debug=False, num_devices=B)

    headT_d = nc.dram_tensor("headT", [P, ND * S], F16, kind="ExternalInput")
    depT_d = nc.dram_tensor("depT", [P, ND * S], F16, kind="ExternalInput")
    u2_d = nc.dram_tensor("u2", [P, ND * D], F16, kind="ExternalInput")
    wd2_d = nc.dram_tensor("wd2", [P, ND], F32, kind="ExternalInput")
    hs2_d = nc.dram_tensor("hs2", [P, NI], F32, kind="ExternalInput")
    out_d = nc.dram_tensor("out", [S, S], F16, kind="ExternalOutput")

    with tile.TileContext(nc) as tc:
        with (
            tc.tile_pool(name="const", bufs=1) as cpool,
            tc.tile_pool(name="persist", bufs=1) as ppool,
            tc.tile_pool(name="outbuf", bufs=3) as outbuf,
            tc.tile_pool(name="hbuf", bufs=2) as hbuf,
            tc.tile_pool(name="ps", bufs=3, space=bass.MemorySpace.PSUM) as ps,
            tc.tile_pool(name="psh", bufs=1, space=bass.MemorySpace.PSUM) as psh,
        ):
            def body():
                # ---- loads: priority order on the ACT HWDGE ring ----
                u2 = cpool.tile([P, ND * D], F16, name="u2", tag="u2")
                nc.scalar.dma_start(u2[:], u2_d[:])
                headT2 = ppool.tile([P, ND * S], F16, name="headT2",
                                    tag="headT2")
                nc.scalar.dma_start(headT2[:], headT_d[:])
                depT2 = ppool.tile([P, ND * S], F16, name="depT2",
                                   tag="depT2")
                for h in range(2):
                    for dc in range(ND):
                        cols = slice(dc * S + h * (S // 2),
                                     dc * S + (h + 1) * (S // 2))
                        nc.scalar.dma_start(depT2[:, cols], depT_d[:, cols])
                headT = [headT2[:, dc * S:(dc + 1) * S] for dc in range(ND)]
                depT = [depT2[:, dc * S:(dc + 1) * S] for dc in range(ND)]

                wd2 = cpool.tile([P, ND], F32, name="wd2", tag="wd2")
                nc.gpsimd.dma_start(wd2[:], wd2_d[:])
                hs2 = cpool.tile([P, NI], F32, name="hs2", tag="hs2")
                nc.gpsimd.dma_start(hs2[:], hs2_d[:])

                # ---- PE warm-up fillers on a memset tile (no DMA deps;
                #      dedicated PSUM tile so nothing waits on eviction) ----
                warm = cpool.tile([P, NB], F16, name="warm", tag="warm")
                nc.vector.memset(warm[:], 0.0)
                pw = psh.tile([P, NB], F32, name="psh", tag="psh")
                for _ in range(NWARM):
                    nc.tensor.matmul(pw[:, 0:512], warm[:, 0:P],
                                     warm[:, 0:512], start=True, stop=True)

                # ---- ATf[e, i] = U^T @ headT + wd (bias in eviction) ----
                atf = [ppool.tile([P, S], F16, name=f"atf{eb}", tag=f"atf{eb}")
                       for eb in range(ND)]

                def atf_chunk(ic):
                    # one [128,1024] 2-bank PSUM tile per (ic, eb), filled
                    # by 2x2 N=512 matmuls, drained by ONE 1024-wide
                    # eviction (ISA caps the moving dim at 512).
                    for eb in range(ND):
                        pa = ps.tile([P, NB], F32, name="ps", tag="ps")
                        for dc in range(ND):
                            for k in range(2):
                                nc.tensor.matmul(
                                    pa[:, k * 512:(k + 1) * 512],
                                    u2[:, dc * D + eb * P:
                                       dc * D + (eb + 1) * P],
                                    headT[dc][:, ic * NB + k * 512:
                                              ic * NB + (k + 1) * 512],
                                    start=(dc == 0), stop=(dc == ND - 1),
                                )
                        dst = atf[eb][:, ic * NB:(ic + 1) * NB]
                        if eb == 0:
                            nc.vector.tensor_scalar_add(
                                dst, pa[:], wd2[:, eb:eb + 1])
                        else:
                            nc.scalar.activation(
                                dst, pa[:], Ident, bias=wd2[:, eb:eb + 1])

                # out row-block: 2 eb x 4 N=512 matmuls into two 2-bank
                # PSUM tiles; one DVE + one ACT 1024-wide eviction per row.
                def out_row(ib):
                    ot = outbuf.tile([P, S], F16, name="ot", tag="ot")
                    pos = [ps.tile([P, NB], F32, name="ps", tag="ps")
                           for _ in range(2)]
                    for eb in range(ND):
                        for c in range(4):
                            nc.tensor.matmul(
                                pos[c // 2][:, (c % 2) * 512:
                                            (c % 2 + 1) * 512],
                                atf[eb][:, ib * P:(ib + 1) * P],
                                depT[eb][:, c * 512:(c + 1) * 512],
                                start=(eb == 0), stop=(eb == ND - 1),
                            )
                    for h in range(2):
                        dst = ot[:, h * NB:(h + 1) * NB]
                        if h == 0:
                            nc.vector.tensor_scalar_add(
                                dst, pos[h][:], hs2[:, ib:ib + 1])
                        else:
                            nc.scalar.activation(
                                dst, pos[h][:], Ident, bias=hs2[:, ib:ib + 1])
                    nc.sync.dma_start(out_d[ib * P:(ib + 1) * P, :], ot[:])

                # Half-row-block for the load-boundary rows: h=0 halves
                # only need the first dep half; h=1 halves run mid-stream.
                def out_half(ib, h):
                    ot = hbuf.tile([P, NB], F16, name="oth", tag="oth")
                    po = ps.tile([P, NB], F32, name="ps", tag="ps")
                    for eb in range(ND):
                        for k in range(2):
                            nc.tensor.matmul(
                                po[:, k * 512:(k + 1) * 512],
                                atf[eb][:, ib * P:(ib + 1) * P],
                                depT[eb][:, h * NB + k * 512:
                                        h * NB + (k + 1) * 512],
                                start=(eb == 0), stop=(eb == ND - 1),
                            )
                    if (ib + h) % 2 == 0:
                        nc.vector.tensor_scalar_add(
                            ot[:], po[:], hs2[:, ib:ib + 1])
                    else:
                        nc.scalar.activation(
                            ot[:], po[:], Ident, bias=hs2[:, ib:ib + 1])
                    nc.sync.dma_start(
                        out_d[ib * P:(ib + 1) * P, h * NB:(h + 1) * NB],
                        ot[:])

                for ic in range(S // NB):
                    atf_chunk(ic)
                for ib in range(3):
                    out_half(ib, 0)
                for ib in range(3, NI):
                    out_row(ib)
                    if ib in (8, 10, 12):
                        out_half((ib - 8) // 2, 1)

            if reps > 1:
                with tc.For_i(0, reps, 1):
                    body()
            else:
                body()

    nc.finalize()
    return nc


_NC_CACHE = {}


def _get_nc(reps=1):
    if reps not in _NC_CACHE:
        _NC_CACHE[reps] = build_nc(reps)
    return _NC_CACHE[reps]


def make_in_maps(head, dep, edge_U, edge_W, edge_b):
    head = np.asarray(head, np.float32)
    dep = np.asarray(dep, np.float32)
    # [B, D, S] -> packed [B, 128, 2*S] with dc-chunks side by side
    headT = np.ascontiguousarray(
        head.astype(np.float16).transpose(0, 2, 1).reshape(B, ND, P, S)
        .transpose(0, 2, 1, 3).reshape(B, P, ND * S))
    depT = np.ascontiguousarray(
        dep.astype(np.float16).transpose(0, 2, 1).reshape(B, ND, P, S)
        .transpose(0, 2, 1, 3).reshape(B, P, ND * S))
    u = np.asarray(edge_U, np.float32).astype(np.float16)
    u2 = np.ascontiguousarray(
        np.concatenate([u[dc * P:(dc + 1) * P, :] for dc in range(ND)],
                       axis=1))                             # [128, 512]
    w = np.asarray(edge_W, np.float32).reshape(-1)
    wh, wd = w[:D], w[D:]
    wd2 = np.ascontiguousarray(wd.reshape(ND, P).T.astype(np.float32))
    b0 = float(np.asarray(edge_b, np.float32).reshape(-1)[0])
    hs = head @ wh + b0                                     # [B, S] f32
    hs2 = np.ascontiguousarray(
        hs.reshape(B, NI, P).transpose(0, 2, 1))            # [B, 128, 16]
    return [
        {"headT": headT[b], "depT": depT[b], "u2": u2, "wd2": wd2,
         "hs2": hs2[b]}
        for b in range(B)
    ]


def kernel(head, dep, edge_U, edge_W, edge_b):
    nc = _get_nc()
    in_maps = make_in_maps(head, dep, edge_U, edge_W, edge_b)
    last_err = None
    for _ in range(3):  # transient device errors happen on this shared env
        try:
            res = run_bass_kernel_spmd(nc, in_maps, core_ids=list(range(B)))
            break
        except Exception as e:  # noqa: BLE001
            last_err = e
    else:
        raise last_err
    return np.stack(
        [res.results[b]["out"].astype(np.float32) for b in range(B)], axis=0)


# revision 28
# speedup vs baseline: 1.0588x; 1.0588x over previous
"""Biaffine edge attention on 8 Trainium2 NeuronCores (fp16, PE-roofline schedule).

Math (per batch b):
    out[i,o] = head[i,:] @ U @ dep[o,:] + head[i,:]@wh + dep[o,:]@wd + b
with head/dep [S=2048, D=256], U [D,D], edge_W = [wh | wd] (each [D]).

Sharding: pure data-parallel over batch B=8 -> one batch per core,
U / edge_W / edge_b replicated. No collectives.

Host prep (layout only + the tiny rank-1 bias):
    headT/depT: inputs pre-transposed to [D, S] fp16 and packed
        [dc0 | dc1] side-by-side as [128, 2S] -- the PE needs the
        contraction dim on partitions for both operands, host-side
        layout beats 64 PE transposes, and the single-DMA packing gives
        8KB DRAM elements (~340 GB/s vs ~220 at 2KB).
    hs2[p, j] = head[j*128+p, :] @ wh + b   (per-row bias, [128, 16] f32)
    u2 = [U[:128, :] | U[128:, :]]          ([128, 512] fp16)
    wd2[p, eb] = wd[eb*128+p]               ([128, 2] f32)

Per-core kernel (fp16 matmuls, f32 PSUM, fp16 stores upcast on host):
    ATf[e,i] = sum_d U[d,e] headT[d,i] + wd[e]    (ds[o] rides the
               e-contraction of the out matmul for free)
    out[i,o] = sum_e ATf[e,i] depT[e,o] + hs2[i]  (bias fused in the
               PSUM->SBUF eviction on ACT/DVE)

Schedule notes:
  - all loads FIFO-serialized on the ACT HWDGE ring in priority order
    (u2, head, dep) -- parallel rings would packet-interleave and delay
    the head bytes the ATf phase waits on; SP ring kept for stores.
  - ~7 junk matmuls on a memset tile warm the PE HAM clock gate
    (1.2 -> 2.4 GHz needs ~3.4us of sustained busy) during the loads.
  - fp16 moving operands stream N=1024 per matmul: out row-blocks are
    2 matmuls x 2 eb into two 2-bank PSUM tiles; halves the PE
    instruction count and the eviction count vs 512-chunks.
  - epilogue: one [128,1024] eviction on DVE + one on ACT per row;
    row bias (hs2) and ATf bias (wd2) ride the eviction for free.
  - rows 0-2 split in halves: their first halves only need the first
    half of dep, filling the PE window while the dep tail is still on
    the DMA ring; second halves run mid-stream.
  - stores: one [128,2048] fp16 DMA per row-block on the SP ring.
"""

import numpy as np

import concourse.bass as bass
import concourse.tile as tile
from concourse import bacc, mybir
from concourse.bass_utils import run_bass_kernel_spmd

B, S, D = 8, 2048, 256
P = 128          # partitions
NB = 1024        # matmul moving free-dim block (two PSUM banks of fp32)
NI = S // P      # 16 output row blocks
ND = D // P      # 2 contraction chunks
NWARM = 4        # PE warm-up filler matmuls (cover load latency, warm HAM)
F32 = mybir.dt.float32
F16 = mybir.dt.float16

Ident = mybir.ActivationFunctionType.Identity


def build_nc(reps=1):
    """reps>1 wraps the body in a HW For_i loop -- used only for timing."""
    nc = bacc.Bacc("TRN2", target_bir_lowering=False, debug=False, num_devices=B)

    headT_d = nc.dram_tensor("headT", [P, ND * S], F16, kind="ExternalInput")
    depT_d = nc.dram_tensor("depT", [P, ND * S], F16, kind="ExternalInput")
    u2_d = nc.dram_tensor("u2", [P, ND * D], F16, kind="ExternalInput")
    wd2_d = nc.dram_tensor("wd2", [P, ND], F32, kind="ExternalInput")
    hs2_d = nc.dram_tensor("hs2", [P, NI], F32, kind="ExternalInput")
    out_d = nc.dram_tensor("out", [S, S], F16, kind="ExternalOutput")

    with tile.TileContext(nc) as tc:
        with (
            tc.tile_pool(name="const", bufs=1) as cpool,
            tc.tile_pool(name="persist", bufs=1) as ppool,
            tc.tile_pool(name="outbuf", bufs=3) as outbuf,
            tc.tile_pool(name="hbuf", bufs=2) as hbuf,
            tc.tile_pool(name="ps", bufs=4, space=bass.MemorySpace.PSUM) as ps,
        ):
            def body():
                # ---- loads: priority order on the ACT HWDGE ring.
                # The DRAM packing interleaves the dc chunks at half-S
                # granularity: [dc0 h0 | dc1 h0 | dc0 h1 | dc1 h1], so each
                # 512KB transfer (4KB DRAM elements, near line rate)
                # unlocks the next compute stage: head-h0 -> ATf ic0,
                # head-h1 -> ic1, dep-h0 -> boundary halves, dep-h1 ->
                # full rows. ----
                u2 = cpool.tile([P, ND * D], F16, name="u2", tag="u2")
                nc.scalar.dma_start(u2[:], u2_d[:])
                headT2 = ppool.tile([P, ND * S], F16, name="headT2",
                                    tag="headT2")
                depT2 = ppool.tile([P, ND * S], F16, name="depT2",
                                   tag="depT2")
                for h in range(2):
                    cols = slice(h * S, (h + 1) * S)
                    nc.scalar.dma_start(headT2[:, cols], headT_d[:, cols])
                for h in range(2):
                    cols = slice(h * S, (h + 1) * S)
                    nc.scalar.dma_start(depT2[:, cols], depT_d[:, cols])

                def hcol(dc, ic, k):
                    # head col of 512-block k within i-chunk ic, d-chunk dc
                    return ic * 2 * NB + dc * NB + k * 512

                def dcol(eb, c):
                    # dep col of 512-wide o-chunk c, e-chunk eb
                    return (c // 2) * 2 * NB + eb * NB + (c % 2) * 512

                wd2 = cpool.tile([P, ND], F32, name="wd2", tag="wd2")
                nc.gpsimd.dma_start(wd2[:], wd2_d[:])
                hs2 = cpool.tile([P, NI], F32, name="hs2", tag="hs2")
                nc.gpsimd.dma_start(hs2[:], hs2_d[:])

                # ---- PE warm-up fillers on a memset tile (no DMA deps;
                #      PE->PE WAW on pool tiles is program-order-free) ----
                warm = cpool.tile([P, 512], F16, name="warm", tag="warm")
                nc.vector.memset(warm[:], 0.0)
                for _ in range(NWARM):
                    pw = ps.tile([P, NB], F32, name="ps", tag="ps")
                    nc.tensor.matmul(pw[:, 0:512], warm[:, 0:P],
                                     warm[:], start=True, stop=True)

                # ---- ATf[e, i] = U^T @ headT + wd (bias in eviction) ----
                atf = [ppool.tile([P, S], F16, name=f"atf{eb}", tag=f"atf{eb}")
                       for eb in range(ND)]

                def atf_chunk(ic):
                    # one [128,1024] 2-bank PSUM tile per (ic, eb), filled
                    # by 2x2 N=512 matmuls, drained by ONE 1024-wide
                    # eviction (ISA caps the moving dim at 512).
                    for eb in range(ND):
                        pa = ps.tile([P, NB], F32, name="ps", tag="ps")
                        for dc in range(ND):
                            for k in range(2):
                                nc.tensor.matmul(
                                    pa[:, k * 512:(k + 1) * 512],
                                    u2[:, dc * D + eb * P:
                                       dc * D + (eb + 1) * P],
                                    headT2[:, hcol(dc, ic, k):
                                           hcol(dc, ic, k) + 512],
                                    start=(dc == 0), stop=(dc == ND - 1),
                                )
                        dst = atf[eb][:, ic * NB:(ic + 1) * NB]
                        if eb == 0:
                            nc.vector.tensor_scalar_add(
                                dst, pa[:], wd2[:, eb:eb + 1])
                        else:
                            nc.scalar.activation(
                                dst, pa[:], Ident, bias=wd2[:, eb:eb + 1])

                # out row-block: 2 eb x 4 N=512 matmuls into two 2-bank
                # PSUM tiles; one DVE + one ACT 1024-wide eviction per row.
                def out_row(ib):
                    ot = outbuf.tile([P, S], F16, name="ot", tag="ot")
                    pos = [ps.tile([P, NB], F32, name="ps", tag="ps")
                           for _ in range(2)]
                    for eb in range(ND):
                        for c in range(4):
                            nc.tensor.matmul(
                                pos[c // 2][:, (c % 2) * 512:
                                            (c % 2 + 1) * 512],
                                atf[eb][:, ib * P:(ib + 1) * P],
                                depT2[:, dcol(eb, c):dcol(eb, c) + 512],
                                start=(eb == 0), stop=(eb == ND - 1),
                            )
                    for h in range(2):
                        dst = ot[:, h * NB:(h + 1) * NB]
                        if h == 0:
                            nc.vector.tensor_scalar_add(
                                dst, pos[h][:], hs2[:, ib:ib + 1])
                        else:
                            nc.scalar.activation(
                                dst, pos[h][:], Ident, bias=hs2[:, ib:ib + 1])
                    nc.sync.dma_start(out_d[ib * P:(ib + 1) * P, :], ot[:])

                # Half-row-block for the load-boundary rows: h=0 halves
                # only need the first dep half; h=1 halves run mid-stream.
                def out_half(ib, h):
                    ot = hbuf.tile([P, NB], F16, name="oth", tag="oth")
                    po = ps.tile([P, NB], F32, name="ps", tag="ps")
                    for eb in range(ND):
                        for k in range(2):
                            c = 2 * h + k
                            nc.tensor.matmul(
                                po[:, k * 512:(k + 1) * 512],
                                atf[eb][:, ib * P:(ib + 1) * P],
                                depT2[:, dcol(eb, c):dcol(eb, c) + 512],
                                start=(eb == 0), stop=(eb == ND - 1),
                            )
                    if (ib + h) % 2 == 0:
                        nc.vector.tensor_scalar_add(
                            ot[:], po[:], hs2[:, ib:ib + 1])
                    else:
                        nc.scalar.activation(
                            ot[:], po[:], Ident, bias=hs2[:, ib:ib + 1])
                    nc.sync.dma_start(
                        out_d[ib * P:(ib + 1) * P, h * NB:(h + 1) * NB],
                        ot[:])

                # rows 0-1 split at the dep-h0/h1 load boundary; the last
                # row is also split so the final store is small.
                atf_chunk(0)
                atf_chunk(1)
                out_half(0, 0)
                out_half(1, 0)
                for ib in range(2, NI - 1):
                    out_row(ib)
                    if ib == 8:
                        out_half(0, 1)
                    elif ib == 10:
                        out_half(1, 1)
                out_half(NI - 1, 0)
                out_half(NI - 1, 1)

            if reps > 1:
                with tc.For_i(0, reps, 1):
                    body()
            else:
                body()

    nc.finalize()
    return nc


_NC_CACHE = {}


def _get_nc(reps=1):
    if reps not in _NC_CACHE:
        _NC_CACHE[reps] = build_nc(reps)
    return _NC_CACHE[reps]


def make_in_maps(head, dep, edge_U, edge_W, edge_b):
    head = np.asarray(head, np.float32)
    dep = np.asarray(dep, np.float32)
    # [B, D, S] -> packed [B, 128, 2*S] interleaving the d-chunks at
    # half-S granularity: cols = [dc0 h0 | dc1 h0 | dc0 h1 | dc1 h1]
    def pack(x):
        t = x.astype(np.float16).transpose(0, 2, 1)     # [B, D, S]
        t = t.reshape(B, ND, P, 2, S // 2)              # dc, p, h, s
        t = t.transpose(0, 2, 3, 1, 4)                  # p, h, dc, s
        return np.ascontiguousarray(t.reshape(B, P, ND * S))

    headT = pack(head)
    depT = pack(dep)
    u = np.asarray(edge_U, np.float32).astype(np.float16)
    u2 = np.ascontiguousarray(
        np.concatenate([u[dc * P:(dc + 1) * P, :] for dc in range(ND)],
                       axis=1))                             # [128, 512]
    w = np.asarray(edge_W, np.float32).reshape(-1)
    wh, wd = w[:D], w[D:]
    wd2 = np.ascontiguousarray(wd.reshape(ND, P).T.astype(np.float32))
    b0 = float(np.asarray(edge_b, np.float32).reshape(-1)[0])
    hs = head @ wh + b0                                     # [B, S] f32
    hs2 = np.ascontiguousarray(
        hs.reshape(B, NI, P).transpose(0, 2, 1))            # [B, 128, 16]
    return [
        {"headT": headT[b], "depT": depT[b], "u2": u2, "wd2": wd2,
         "hs2": hs2[b]}
        for b in range(B)
    ]


def kernel(head, dep, edge_U, edge_W, edge_b):
    nc = _get_nc()
    in_maps = make_in_maps(head, dep, edge_U, edge_W, edge_b)
    last_err = None
    for _ in range(3):  # transient device errors happen on this shared env
        try:
            res = run_bass_kernel_spmd(nc, in_maps, core_ids=list(range(B)))
            break
        except Exception as e:  # noqa: BLE001
            last_err = e
    else:
        raise last_err
    return np.stack(
        [res.results[b]["out"].astype(np.float32) for b in range(B)], axis=0)


# revision 31
# speedup vs baseline: 1.0972x; 1.0363x over previous
"""Biaffine edge attention on 8 Trainium2 NeuronCores (fp16, PE-roofline schedule).

Math (per batch b):
    out[i,o] = head[i,:] @ U @ dep[o,:] + head[i,:]@wh + dep[o,:]@wd + b
with head/dep [S=2048, D=256], U [D,D], edge_W = [wh | wd] (each [D]).

Sharding: pure data-parallel over batch B=8 -> one batch per core,
U / edge_W / edge_b replicated. No collectives.

Host prep (layout only + the tiny rank-1 bias):
    headT/depT: inputs pre-transposed to [D, S] fp16 and packed
        [dc0 | dc1] side-by-side as [128, 2S] -- the PE needs the
        contraction dim on partitions for both operands, host-side
        layout beats 64 PE transposes, and the single-DMA packing gives
        8KB DRAM elements (~340 GB/s vs ~220 at 2KB).
    hs2[p, j] = head[j*128+p, :] @ wh + b   (per-row bias, [128, 16] f32)
    u2 = [U[:128, :] | U[128:, :]]          ([128, 512] fp16)
    wd2[p, eb] = wd[eb*128+p]               ([128, 2] f32)

Per-core kernel (fp16 matmuls, f32 PSUM, fp16 stores upcast on host):
    ATf[e,i] = sum_d U[d,e] headT[d,i] + wd[e]    (ds[o] rides the
               e-contraction of the out matmul for free)
    out[i,o] = sum_e ATf[e,i] depT[e,o] + hs2[i]  (bias fused in the
               PSUM->SBUF eviction on ACT/DVE)

Schedule notes:
  - all loads FIFO-serialized on the ACT HWDGE ring in priority order
    (u2, head, dep) -- parallel rings would packet-interleave and delay
    the head bytes the ATf phase waits on; SP ring kept for stores.
  - ~7 junk matmuls on a memset tile warm the PE HAM clock gate
    (1.2 -> 2.4 GHz needs ~3.4us of sustained busy) during the loads.
  - fp16 moving operands stream N=1024 per matmul: out row-blocks are
    2 matmuls x 2 eb into two 2-bank PSUM tiles; halves the PE
    instruction count and the eviction count vs 512-chunks.
  - epilogue: one [128,1024] eviction on DVE + one on ACT per row;
    row bias (hs2) and ATf bias (wd2) ride the eviction for free.
  - rows 0-2 split in halves: their first halves only need the first
    half of dep, filling the PE window while the dep tail is still on
    the DMA ring; second halves run mid-stream.
  - stores: one [128,2048] fp16 DMA per row-block on the SP ring.
"""

import numpy as np

import concourse.bass as bass
import concourse.tile as tile
from concourse import bacc, mybir
from concourse.bass_utils import run_bass_kernel_spmd

B, S, D = 8, 2048, 256
P = 128          # partitions
NB = 1024        # matmul moving free-dim block (two PSUM banks of fp32)
NI = S // P      # 16 output row blocks
ND = D // P      # 2 contraction chunks
NWARM = 4        # PE warm-up filler matmuls (cover load latency, warm HAM)
F32 = mybir.dt.float32
F16 = mybir.dt.float16

Ident = mybir.ActivationFunctionType.Identity


def build_nc(reps=1):
    """reps>1 wraps the body in a HW For_i loop -- used only for timing."""
    nc = bacc.Bacc("TRN2", target_bir_lowering=False, debug=False, num_devices=B)

    # chunk-major [4, 128, 1024]: chunk j = (h, dc) is a fully contiguous
    # 256KB block, so each load DMA is a sequential DRAM stream (line rate)
    headT_d = nc.dram_tensor("headT", [2 * ND, P, S // 2], F16,
                             kind="ExternalInput")
    depT_d = nc.dram_tensor("depT", [2 * ND, P, S // 2], F16,
                            kind="ExternalInput")
    u2_d = nc.dram_tensor("u2", [P, ND * D], F16, kind="ExternalInput")
    wd2_d = nc.dram_tensor("wd2", [P, ND], F32, kind="ExternalInput")
    hs2_d = nc.dram_tensor("hs2", [P, NI], F32, kind="ExternalInput")
    out_d = nc.dram_tensor("out", [S, S], F16, kind="ExternalOutput")

    with tile.TileContext(nc) as tc:
        with (
            tc.tile_pool(name="const", bufs=1) as cpool,
            tc.tile_pool(name="persist", bufs=1) as ppool,
            tc.tile_pool(name="outbuf", bufs=3) as outbuf,
            tc.tile_pool(name="hbuf", bufs=2) as hbuf,
            tc.tile_pool(name="ps", bufs=4, space=bass.MemorySpace.PSUM) as ps,
        ):
            def body():
                # ---- loads: priority order on the ACT HWDGE ring.
                # The DRAM packing interleaves the dc chunks at half-S
                # granularity: [dc0 h0 | dc1 h0 | dc0 h1 | dc1 h1], so each
                # 512KB transfer (4KB DRAM elements, near line rate)
                # unlocks the next compute stage: head-h0 -> ATf ic0,
                # head-h1 -> ic1, dep-h0 -> boundary halves, dep-h1 ->
                # full rows. ----
                u2 = cpool.tile([P, ND * D], F16, name="u2", tag="u2")
                nc.scalar.dma_start(u2[:], u2_d[:])
                headT2 = ppool.tile([P, ND * S], F16, name="headT2",
                                    tag="headT2")
                depT2 = ppool.tile([P, ND * S], F16, name="depT2",
                                   tag="depT2")
                wd2 = cpool.tile([P, ND], F32, name="wd2", tag="wd2")
                hs2 = cpool.tile([P, NI], F32, name="hs2", tag="hs2")
                for j in range(2 * ND):
                    nc.scalar.dma_start(
                        headT2[:, j * NB:(j + 1) * NB], headT_d[j])
                nc.scalar.dma_start(wd2[:], wd2_d[:])
                for j in range(2):
                    nc.scalar.dma_start(
                        depT2[:, j * NB:(j + 1) * NB], depT_d[j])
                nc.scalar.dma_start(hs2[:], hs2_d[:])
                for j in range(2, 2 * ND):
                    nc.scalar.dma_start(
                        depT2[:, j * NB:(j + 1) * NB], depT_d[j])

                def hcol(dc, ic, k):
                    # head col of 512-block k within i-chunk ic, d-chunk dc
                    return ic * 2 * NB + dc * NB + k * 512

                def dcol(eb, c):
                    # dep col of 512-wide o-chunk c, e-chunk eb
                    return (c // 2) * 2 * NB + eb * NB + (c % 2) * 512

                # ---- PE warm-up fillers on a memset tile (no DMA deps;
                #      PE->PE WAW on pool tiles is program-order-free) ----
                warm = cpool.tile([P, 512], F16, name="warm", tag="warm")
                nc.vector.memset(warm[:], 0.0)
                for _ in range(NWARM):
                    pw = ps.tile([P, NB], F32, name="ps", tag="ps")
                    nc.tensor.matmul(pw[:, 0:512], warm[:, 0:P],
                                     warm[:], start=True, stop=True)

                # ---- ATf[e, i] = U^T @ headT + wd (bias in eviction) ----
                atf = [ppool.tile([P, S], F16, name=f"atf{eb}", tag=f"atf{eb}")
                       for eb in range(ND)]

                def atf_chunk(ic):
                    # one [128,1024] 2-bank PSUM tile per (ic, eb), filled
                    # by 2x2 N=512 matmuls, drained by ONE 1024-wide
                    # eviction (ISA caps the moving dim at 512).
                    for eb in range(ND):
                        pa = ps.tile([P, NB], F32, name="ps", tag="ps")
                        for dc in range(ND):
                            for k in range(2):
                                nc.tensor.matmul(
                                    pa[:, k * 512:(k + 1) * 512],
                                    u2[:, dc * D + eb * P:
                                       dc * D + (eb + 1) * P],
                                    headT2[:, hcol(dc, ic, k):
                                           hcol(dc, ic, k) + 512],
                                    start=(dc == 0), stop=(dc == ND - 1),
                                )
                        dst = atf[eb][:, ic * NB:(ic + 1) * NB]
                        if eb == 0:
                            nc.vector.tensor_scalar_add(
                                dst, pa[:], wd2[:, eb:eb + 1])
                        else:
                            nc.scalar.activation(
                                dst, pa[:], Ident, bias=wd2[:, eb:eb + 1])

                # out row-block: 2 eb x 4 N=512 matmuls into two 2-bank
                # PSUM tiles; one DVE + one ACT 1024-wide eviction per row.
                def out_row(ib):
                    ot = outbuf.tile([P, S], F16, name="ot", tag="ot")
                    pos = [ps.tile([P, NB], F32, name="ps", tag="ps")
                           for _ in range(2)]
                    for eb in range(ND):
                        for c in range(4):
                            nc.tensor.matmul(
                                pos[c // 2][:, (c % 2) * 512:
                                            (c % 2 + 1) * 512],
                                atf[eb][:, ib * P:(ib + 1) * P],
                                depT2[:, dcol(eb, c):dcol(eb, c) + 512],
                                start=(eb == 0), stop=(eb == ND - 1),
                            )
                    for h in range(2):
                        dst = ot[:, h * NB:(h + 1) * NB]
                        if h == 0:
                            nc.vector.tensor_scalar_add(
                                dst, pos[h][:], hs2[:, ib:ib + 1])
                        else:
                            nc.scalar.activation(
                                dst, pos[h][:], Ident, bias=hs2[:, ib:ib + 1])
                    nc.sync.dma_start(out_d[ib * P:(ib + 1) * P, :], ot[:])

                # Half-row-block for the load-boundary rows: h=0 halves
                # only need the first dep half; h=1 halves run mid-stream.
                def out_half(ib, h):
                    ot = hbuf.tile([P, NB], F16, name="oth", tag="oth")
                    po = ps.tile([P, NB], F32, name="ps", tag="ps")
                    for eb in range(ND):
                        for k in range(2):
                            c = 2 * h + k
                            nc.tensor.matmul(
                                po[:, k * 512:(k + 1) * 512],
                                atf[eb][:, ib * P:(ib + 1) * P],
                                depT2[:, dcol(eb, c):dcol(eb, c) + 512],
                                start=(eb == 0), stop=(eb == ND - 1),
                            )
                    if (ib + h) % 2 == 0:
                        nc.vector.tensor_scalar_add(
                            ot[:], po[:], hs2[:, ib:ib + 1])
                    else:
                        nc.scalar.activation(
                            ot[:], po[:], Ident, bias=hs2[:, ib:ib + 1])
                    nc.sync.dma_start(
                        out_d[ib * P:(ib + 1) * P, h * NB:(h + 1) * NB],
                        ot[:])

                # rows 0-1 split at the dep-h0/h1 load boundary; the last
                # row is also split so the final store is small.
                atf_chunk(0)
                atf_chunk(1)
                out_half(0, 0)
                out_half(1, 0)
                for ib in range(2, NI - 1):
                    out_row(ib)
                    if ib == 8:
                        out_half(0, 1)
                    elif ib == 10:
                        out_half(1, 1)
                out_half(NI - 1, 0)
                out_half(NI - 1, 1)

            if reps > 1:
                with tc.For_i(0, reps, 1):
                    body()
            else:
                body()

    nc.finalize()
    return nc


_NC_CACHE = {}


def _get_nc(reps=1):
    if reps not in _NC_CACHE:
        _NC_CACHE[reps] = build_nc(reps)
    return _NC_CACHE[reps]


def make_in_maps(head, dep, edge_U, edge_W, edge_b):
    head = np.asarray(head, np.float32)
    dep = np.asarray(dep, np.float32)
    # [B, D, S] -> chunk-major [B, 4, 128, 1024]: chunk j=(h, dc) holds
    # i-cols of half h for d-chunk dc, each chunk contiguous in DRAM
    def pack(x):
        t = x.astype(np.float16).transpose(0, 2, 1)     # [B, D, S]
        t = t.reshape(B, ND, P, 2, S // 2)              # dc, p, h, s
        t = t.transpose(0, 3, 1, 2, 4)                  # h, dc, p, s
        return np.ascontiguousarray(t.reshape(B, 2 * ND, P, S // 2))

    headT = pack(head)
    depT = pack(dep)
    u = np.asarray(edge_U, np.float32).astype(np.float16)
    u2 = np.ascontiguousarray(
        np.concatenate([u[dc * P:(dc + 1) * P, :] for dc in range(ND)],
                       axis=1))                             # [128, 512]
    w = np.asarray(edge_W, np.float32).reshape(-1)
    wh, wd = w[:D], w[D:]
    wd2 = np.ascontiguousarray(wd.reshape(ND, P).T.astype(np.float32))
    b0 = float(np.asarray(edge_b, np.float32).reshape(-1)[0])
    hs = head @ wh + b0                                     # [B, S] f32
    hs2 = np.ascontiguousarray(
        hs.reshape(B, NI, P).transpose(0, 2, 1))            # [B, 128, 16]
    return [
        {"headT": headT[b], "depT": depT[b], "u2": u2, "wd2": wd2,
         "hs2": hs2[b]}
        for b in range(B)
    ]


def kernel(head, dep, edge_U, edge_W, edge_b):
    nc = _get_nc()
    in_maps = make_in_maps(head, dep, edge_U, edge_W, edge_b)
    last_err = None
    for _ in range(3):  # transient device errors happen on this shared env
        try:
            res = run_bass_kernel_spmd(nc, in_maps, core_ids=list(range(B)))
            break
        except Exception as e:  # noqa: BLE001
            last_err = e
    else:
        raise last_err
    return np.stack(
        [res.results[b]["out"].astype(np.float32) for b in range(B)], axis=0)


# revision 32
# speedup vs baseline: 1.1293x; 1.0293x over previous
"""Biaffine edge attention on 8 Trainium2 NeuronCores (fp16, PE-roofline schedule).

Math (per batch b):
    out[i,o] = head[i,:] @ U @ dep[o,:] + head[i,:]@wh + dep[o,:]@wd + b
with head/dep [S=2048, D=256], U [D,D], edge_W = [wh | wd] (each [D]).

Sharding: pure data-parallel over batch B=8 -> one batch per core,
U / edge_W / edge_b replicated. No collectives.

Host prep (layout only + the tiny rank-1 bias):
    headT/depT: inputs pre-transposed to [D, S] fp16 and packed
        [dc0 | dc1] side-by-side as [128, 2S] -- the PE needs the
        contraction dim on partitions for both operands, host-side
        layout beats 64 PE transposes, and the single-DMA packing gives
        8KB DRAM elements (~340 GB/s vs ~220 at 2KB).
    hs2[p, j] = head[j*128+p, :] @ wh + b   (per-row bias, [128, 16] f32)
    u2 = [U[:128, :] | U[128:, :]]          ([128, 512] fp16)
    wd2[p, eb] = wd[eb*128+p]               ([128, 2] f32)

Per-core kernel (fp16 matmuls, f32 PSUM, fp16 stores upcast on host):
    ATf[e,i] = sum_d U[d,e] headT[d,i] + wd[e]    (ds[o] rides the
               e-contraction of the out matmul for free)
    out[i,o] = sum_e ATf[e,i] depT[e,o] + hs2[i]  (bias fused in the
               PSUM->SBUF eviction on ACT/DVE)

Schedule notes:
  - all loads FIFO-serialized on the ACT HWDGE ring in priority order
    (u2, head, dep) -- parallel rings would packet-interleave and delay
    the head bytes the ATf phase waits on; SP ring kept for stores.
  - ~7 junk matmuls on a memset tile warm the PE HAM clock gate
    (1.2 -> 2.4 GHz needs ~3.4us of sustained busy) during the loads.
  - fp16 moving operands stream N=1024 per matmul: out row-blocks are
    2 matmuls x 2 eb into two 2-bank PSUM tiles; halves the PE
    instruction count and the eviction count vs 512-chunks.
  - epilogue: one [128,1024] eviction on DVE + one on ACT per row;
    row bias (hs2) and ATf bias (wd2) ride the eviction for free.
  - rows 0-2 split in halves: their first halves only need the first
    half of dep, filling the PE window while the dep tail is still on
    the DMA ring; second halves run mid-stream.
  - stores: one [128,2048] fp16 DMA per row-block on the SP ring.
"""

import numpy as np

import concourse.bass as bass
import concourse.tile as tile
from concourse import bacc, mybir
from concourse.bass_utils import run_bass_kernel_spmd

B, S, D = 8, 2048, 256
P = 128          # partitions
NB = 1024        # matmul moving free-dim block (two PSUM banks of fp32)
NI = S // P      # 16 output row blocks
ND = D // P      # 2 contraction chunks
NWARM = 6        # PE warm-up filler matmuls (cover load latency, warm HAM)
F32 = mybir.dt.float32
F16 = mybir.dt.float16

Ident = mybir.ActivationFunctionType.Identity


def build_nc(reps=1):
    """reps>1 wraps the body in a HW For_i loop -- used only for timing."""
    nc = bacc.Bacc("TRN2", target_bir_lowering=False, debug=False, num_devices=B)

    # chunk-major [4, 128, 1024]: chunk j = (h, dc) is a fully contiguous
    # 256KB block, so each load DMA is a sequential DRAM stream (line rate)
    headT_d = nc.dram_tensor("headT", [2 * ND, P, S // 2], F16,
                             kind="ExternalInput")
    depT_d = nc.dram_tensor("depT", [2 * ND, P, S // 2], F16,
                            kind="ExternalInput")
    u2_d = nc.dram_tensor("u2", [P, ND * D], F16, kind="ExternalInput")
    wd2_d = nc.dram_tensor("wd2", [P, ND], F32, kind="ExternalInput")
    hs2_d = nc.dram_tensor("hs2", [P, NI], F32, kind="ExternalInput")
    out_d = nc.dram_tensor("out", [S, S], F16, kind="ExternalOutput")

    with tile.TileContext(nc) as tc:
        with (
            tc.tile_pool(name="const", bufs=1) as cpool,
            tc.tile_pool(name="persist", bufs=1) as ppool,
            tc.tile_pool(name="outbuf", bufs=3) as outbuf,
            tc.tile_pool(name="hbuf", bufs=2) as hbuf,
            tc.tile_pool(name="ps", bufs=4, space=bass.MemorySpace.PSUM) as ps,
        ):
            def body():
                # ---- loads: priority order on the ACT HWDGE ring.
                # The DRAM packing interleaves the dc chunks at half-S
                # granularity: [dc0 h0 | dc1 h0 | dc0 h1 | dc1 h1], so each
                # 512KB transfer (4KB DRAM elements, near line rate)
                # unlocks the next compute stage: head-h0 -> ATf ic0,
                # head-h1 -> ic1, dep-h0 -> boundary halves, dep-h1 ->
                # full rows. ----
                u2 = cpool.tile([P, ND * D], F16, name="u2", tag="u2")
                nc.scalar.dma_start(u2[:], u2_d[:])
                headT2 = ppool.tile([P, ND * S], F16, name="headT2",
                                    tag="headT2")
                depT2 = ppool.tile([P, ND * S], F16, name="depT2",
                                   tag="depT2")
                wd2 = cpool.tile([P, ND], F32, name="wd2", tag="wd2")
                hs2 = cpool.tile([P, NI], F32, name="hs2", tag="hs2")
                for j in range(2 * ND):
                    nc.scalar.dma_start(
                        headT2[:, j * NB:(j + 1) * NB], headT_d[j])
                nc.scalar.dma_start(wd2[:], wd2_d[:])
                for j in range(2):
                    nc.scalar.dma_start(
                        depT2[:, j * NB:(j + 1) * NB], depT_d[j])
                nc.scalar.dma_start(hs2[:], hs2_d[:])
                for j in range(2, 2 * ND):
                    nc.scalar.dma_start(
                        depT2[:, j * NB:(j + 1) * NB], depT_d[j])

                def hcol(dc, ic, k):
                    # head col of 512-block k within i-chunk ic, d-chunk dc
                    return ic * 2 * NB + dc * NB + k * 512

                def dcol(eb, c):
                    # dep col of 512-wide o-chunk c, e-chunk eb
                    return (c // 2) * 2 * NB + eb * NB + (c % 2) * 512

                # ---- PE warm-up fillers on a memset tile (no DMA deps;
                #      PE->PE WAW on pool tiles is program-order-free) ----
                warm = cpool.tile([P, 512], F16, name="warm", tag="warm")
                nc.vector.memset(warm[:], 0.0)
                for _ in range(NWARM):
                    pw = ps.tile([P, NB], F32, name="ps", tag="ps")
                    nc.tensor.matmul(pw[:, 0:512], warm[:, 0:P],
                                     warm[:], start=True, stop=True)

                # ---- ATf[e, i] = U^T @ headT + wd (bias in eviction) ----
                atf = [ppool.tile([P, S], F16, name=f"atf{eb}", tag=f"atf{eb}")
                       for eb in range(ND)]

                def atf_chunk(ic):
                    # one [128,1024] 2-bank PSUM tile per (ic, eb), filled
                    # by 2x2 N=512 matmuls, drained by ONE 1024-wide
                    # eviction (ISA caps the moving dim at 512).
                    for eb in range(ND):
                        pa = ps.tile([P, NB], F32, name="ps", tag="ps")
                        for dc in range(ND):
                            for k in range(2):
                                nc.tensor.matmul(
                                    pa[:, k * 512:(k + 1) * 512],
                                    u2[:, dc * D + eb * P:
                                       dc * D + (eb + 1) * P],
                                    headT2[:, hcol(dc, ic, k):
                                           hcol(dc, ic, k) + 512],
                                    start=(dc == 0), stop=(dc == ND - 1),
                                )
                        dst = atf[eb][:, ic * NB:(ic + 1) * NB]
                        if eb == 0:
                            nc.vector.tensor_scalar_add(
                                dst, pa[:], wd2[:, eb:eb + 1])
                        else:
                            nc.scalar.activation(
                                dst, pa[:], Ident, bias=wd2[:, eb:eb + 1])

                # out row-block: 2 eb x 4 N=512 matmuls into two 2-bank
                # PSUM tiles; one DVE + one ACT 1024-wide eviction per row.
                def out_row(ib):
                    ot = outbuf.tile([P, S], F16, name="ot", tag="ot")
                    pos = [ps.tile([P, NB], F32, name="ps", tag="ps")
                           for _ in range(2)]
                    for eb in range(ND):
                        for c in range(4):
                            nc.tensor.matmul(
                                pos[c // 2][:, (c % 2) * 512:
                                            (c % 2 + 1) * 512],
                                atf[eb][:, ib * P:(ib + 1) * P],
                                depT2[:, dcol(eb, c):dcol(eb, c) + 512],
                                start=(eb == 0), stop=(eb == ND - 1),
                            )
                    for h in range(2):
                        dst = ot[:, h * NB:(h + 1) * NB]
                        if h == 0:
                            nc.vector.tensor_scalar_add(
                                dst, pos[h][:], hs2[:, ib:ib + 1])
                        else:
                            nc.scalar.activation(
                                dst, pos[h][:], Ident, bias=hs2[:, ib:ib + 1])
                    nc.sync.dma_start(out_d[ib * P:(ib + 1) * P, :], ot[:])

                # Half-row-block for the load-boundary rows: h=0 halves
                # only need the first dep half; h=1 halves run mid-stream.
                def out_half(ib, h):
                    ot = hbuf.tile([P, NB], F16, name="oth", tag="oth")
                    po = ps.tile([P, NB], F32, name="ps", tag="ps")
                    for eb in range(ND):
                        for k in range(2):
                            c = 2 * h + k
                            nc.tensor.matmul(
                                po[:, k * 512:(k + 1) * 512],
                                atf[eb][:, ib * P:(ib + 1) * P],
                                depT2[:, dcol(eb, c):dcol(eb, c) + 512],
                                start=(eb == 0), stop=(eb == ND - 1),
                            )
                    if (ib + h) % 2 == 0:
                        nc.vector.tensor_scalar_add(
                            ot[:], po[:], hs2[:, ib:ib + 1])
                    else:
                        nc.scalar.activation(
                            ot[:], po[:], Ident, bias=hs2[:, ib:ib + 1])
                    nc.sync.dma_start(
                        out_d[ib * P:(ib + 1) * P, h * NB:(h + 1) * NB],
                        ot[:])

                # rows 0-1 split at the dep-h0/h1 load boundary; the last
                # row is also split so the final store is small.
                atf_chunk(0)
                atf_chunk(1)
                out_half(0, 0)
                out_half(1, 0)
                for ib in range(2, NI - 1):
                    out_row(ib)
                    if ib == 8:
                        out_half(0, 1)
                    elif ib == 10:
                        out_half(1, 1)
                out_half(NI - 1, 0)
                out_half(NI - 1, 1)

            if reps > 1:
                with tc.For_i(0, reps, 1):
                    body()
            else:
                body()

    nc.finalize()
    return nc


_NC_CACHE = {}


def _get_nc(reps=1):
    if reps not in _NC_CACHE:
        _NC_CACHE[reps] = build_nc(reps)
    return _NC_CACHE[reps]


def make_in_maps(head, dep, edge_U, edge_W, edge_b):
    head = np.asarray(head, np.float32)
    dep = np.asarray(dep, np.float32)
    # [B, D, S] -> chunk-major [B, 4, 128, 1024]: chunk j=(h, dc) holds
    # i-cols of half h for d-chunk dc, each chunk contiguous in DRAM
    def pack(x):
        t = x.astype(np.float16).transpose(0, 2, 1)     # [B, D, S]
        t = t.reshape(B, ND, P, 2, S // 2)              # dc, p, h, s
        t = t.transpose(0, 3, 1, 2, 4)                  # h, dc, p, s
        return np.ascontiguousarray(t.reshape(B, 2 * ND, P, S // 2))

    headT = pack(head)
    depT = pack(dep)
    u = np.asarray(edge_U, np.float32).astype(np.float16)
    u2 = np.ascontiguousarray(
        np.concatenate([u[dc * P:(dc + 1) * P, :] for dc in range(ND)],
                       axis=1))                             # [128, 512]
    w = np.asarray(edge_W, np.float32).reshape(-1)
    wh, wd = w[:D], w[D:]
    wd2 = np.ascontiguousarray(wd.reshape(ND, P).T.astype(np.float32))
    b0 = float(np.asarray(edge_b, np.float32).reshape(-1)[0])
    hs = head @ wh + b0                                     # [B, S] f32
    hs2 = np.ascontiguousarray(
        hs.reshape(B, NI, P).transpose(0, 2, 1))            # [B, 128, 16]
    return [
        {"headT": headT[b], "depT": depT[b], "u2": u2, "wd2": wd2,
         "hs2": hs2[b]}
        for b in range(B)
    ]


def kernel(head, dep, edge_U, edge_W, edge_b):
    nc = _get_nc()
    in_maps = make_in_maps(head, dep, edge_U, edge_W, edge_b)
    last_err = None
    for _ in range(3):  # transient device errors happen on this shared env
        try:
            res = run_bass_kernel_spmd(nc, in_maps, core_ids=list(range(B)))
            break
        except Exception as e:  # noqa: BLE001
            last_err = e
    else:
        raise last_err
    return np.stack(
        [res.results[b]["out"].astype(np.float32) for b in range(B)], axis=0)
